# revision 5
# baseline (speedup 1.0000x reference)
"""Distributed Trainium2 kernel for the AEN (attentive episodic network) problem.

Reference computation (shapes):
    support_vs = support @ Wv.T + bv                    [8192, 512]
    q_proto    = queries @ Wv.T + bv                    [8192, 512]
    support_ks = LN(support @ Wk.T + bk)                [8192, 512]
    queries_qs = LN(queries @ Wq.T + bq)                [8192, 512]
    scores     = queries_qs @ support_ks.T / sqrt(512)  [8192, 8192]
    affinity   = softmax(scores, axis=1)
    class_proto= affinity @ support_vs                  [8192, 512]
    returns (q_proto, class_proto)

Sharding: rows (queries AND support) split 1024/core across 8 cores; weights
replicated.  Each core projects its support slice to K/V, AllGathers the
projected K|V (bf16, 2.1MB/core in -> 16.8MB out), and independently computes
its 1024-query slice of the attention.  Query-side projections overlap the
AllGather.  All matmuls in bf16 (f32 PSUM accumulation).

Layouts (on-chip): activations enter feature-major ([d, tok], host
pre-transposed) so projections emit token-major tiles directly; K and the
normalized queries are PE-transposed per 128x128 block into feature-major
for the scores matmul; exp(scores.T) tiles [s,q] then serve directly as lhsT
for both the attention@V matmul (token-major out) and the per-query softmax
denominator (rhs = ones column), so the big [8192 x 1024] probability matrix
is never transposed.
"""

import os

import ml_dtypes
import numpy as np

D = 1024  # model dim
O = 512  # out dim
NCORES = 8
NL = 1024  # rows per core (both queries and support)
NMT = NL // 128  # 8 token tiles per core slice
NDT = D // 128  # 8 contraction tiles
NOT = O // 128  # 4 outdim tiles
NST = NCORES * NMT  # 64 global support tiles
SCALE = 1.0 / float(np.sqrt(np.float32(O)))
LN_EPS = 1e-5
BF16 = ml_dtypes.bfloat16

_CACHE: dict = {}

LAST_EXEC_TIME_NS = None
LAST_RESULTS = None


def _build_graph():
    import concourse.bass as bass  # noqa: F401
    import concourse.tile as tile
    from concourse import bacc, mybir
    from concourse.masks import make_identity

    f32 = mybir.dt.float32
    bf16 = mybir.dt.bfloat16
    Alu = mybir.AluOpType
    Act = mybir.ActivationFunctionType

    nc = bacc.Bacc(
        "TRN2", target_bir_lowering=False, debug=False, num_devices=NCORES
    )

    sT = nc.dram_tensor("sT", [D, NL], bf16, kind="ExternalInput").ap()
    qT = nc.dram_tensor("qT", [D, NL], bf16, kind="ExternalInput").ap()
    w = nc.dram_tensor("w", [D, 3 * O], bf16, kind="ExternalInput").ap()
    bq_b = nc.dram_tensor("bq_b", [128, O], f32, kind="ExternalInput").ap()
    bk_b = nc.dram_tensor("bk_b", [128, O], f32, kind="ExternalInput").ap()
    bv_b = nc.dram_tensor("bv_b", [128, O], f32, kind="ExternalInput").ap()
    g_b = nc.dram_tensor("g_b", [128, O], f32, kind="ExternalInput").ap()
    be_b = nc.dram_tensor("be_b", [128, O], f32, kind="ExternalInput").ap()
    out_q = nc.dram_tensor("out_q", [NL, O], f32, kind="ExternalOutput").ap()
    out_c = nc.dram_tensor("out_c", [NL, O], f32, kind="ExternalOutput").ap()

    KSZ = O * NL  # elements of K_iT block [512, 1024]

    from contextlib import ExitStack

    with tile.TileContext(nc) as tc:
        with ExitStack() as ctx:
            ent = ctx.enter_context
            consts = ent(tc.tile_pool(name="consts", bufs=1))
            wp = ent(tc.tile_pool(name="wp", bufs=NDT))
            actp = ent(tc.tile_pool(name="actp", bufs=2 * NDT))
            lnp = ent(tc.tile_pool(name="lnp", bufs=4))
            stp = ent(tc.tile_pool(name="stp", bufs=10))
            yp = ent(tc.tile_pool(name="yp", bufs=3))
            ktl = ent(tc.tile_pool(name="ktl", bufs=NOT))
            vl = ent(tc.tile_pool(name="vl", bufs=NMT))
            qqp = ent(tc.tile_pool(name="qqp", bufs=NOT))
            ksp = ent(tc.tile_pool(name="ksp", bufs=2 * NOT))
            vsp = ent(tc.tile_pool(name="vsp", bufs=2 * NMT))
            exl = ent(tc.tile_pool(name="exl", bufs=6))
            ocp = ent(tc.tile_pool(name="ocp", bufs=4))
            psA = ent(tc.tile_pool(name="psA", bufs=4, space="PSUM"))
            psB = ent(tc.tile_pool(name="psB", bufs=3, space="PSUM"))
            psS = ent(tc.tile_pool(name="psS", bufs=1, space="PSUM"))
            dram = ent(tc.tile_pool(name="dram", bufs=1, space="DRAM"))
            ident = consts.tile([128, 128], bf16, name="ident")
            make_identity(nc, ident)
            ones = consts.tile([128, 1], bf16, name="ones")
            nc.vector.memset(ones, 1.0)
            eps_t = consts.tile([128, 1], f32, name="eps_t")
            nc.vector.memset(eps_t, LN_EPS)

            bq_sb = consts.tile([128, O], f32, name="bq_sb")
            nc.sync.dma_start(out=bq_sb, in_=bq_b)
            bk_sb = consts.tile([128, O], f32, name="bk_sb")
            nc.sync.dma_start(out=bk_sb, in_=bk_b)
            bv_sb = consts.tile([128, O], f32, name="bv_sb")
            nc.sync.dma_start(out=bv_sb, in_=bv_b)
            g_sb = consts.tile([128, O], f32, name="g_sb")
            nc.sync.dma_start(out=g_sb, in_=g_b)
            be_sb = consts.tile([128, O], f32, name="be_sb")
            nc.sync.dma_start(out=be_sb, in_=be_b)

            wt = []
            for k in range(NDT):
                wtk = wp.tile([128, 3 * O], bf16, name=f"wt{k}", tag="wt")
                nc.sync.dma_start(out=wtk, in_=w[k * 128 : (k + 1) * 128, :])
                wt.append(wtk)
            sTt = []
            for k in range(NDT):
                stk = actp.tile([128, NL], bf16, name=f"sTt{k}", tag="act")
                nc.sync.dma_start(out=stk, in_=sT[k * 128 : (k + 1) * 128, :])
                sTt.append(stk)

            cc_in = dram.tile([NL * D], bf16, name="cc_in")
            cc_out = dram.tile(
                [NCORES * NL * D], bf16, name="cc_out", addr_space="Shared"
            )

            def ln_transpose(ps, bias_sb, dstT, m):
                # bias add -> layernorm -> gamma/beta -> bf16 -> PE-transpose
                # the token tile [128 tok, 512 o] into dstT[j][:, m-slice].
                pre = lnp.tile([128, O], f32, name="pre", tag="lnp")
                nc.vector.tensor_add(pre, ps, bias_sb)
                stats = stp.tile([128, 6], f32, name="stats", tag="stp")
                nc.vector.bn_stats(stats, pre)
                mv = stp.tile([128, 2], f32, name="mv", tag="stp")
                nc.vector.bn_aggr(mv, stats)
                rstd = stp.tile([128, 1], f32, name="rstd", tag="stp")
                nc.scalar.activation(
                    rstd, mv[:, 1:2], Act.Sqrt, bias=eps_t, scale=1.0
                )
                nc.vector.reciprocal(rstd, rstd)
                t1 = lnp.tile([128, O], f32, name="t1", tag="lnp")
                nc.vector.tensor_scalar(
                    t1, pre, mv[:, 0:1], rstd, Alu.subtract, Alu.mult
                )
                t2 = lnp.tile([128, O], f32, name="t2", tag="lnp")
                nc.vector.tensor_mul(t2, t1, g_sb)
                y = yp.tile([128, O], bf16, name="y", tag="yp")
                nc.vector.tensor_add(y, t2, be_sb)
                for j in range(NOT):
                    pt = psB.tile([128, 128], bf16, name="pt", tag="psB")
                    nc.tensor.transpose(pt, y[:, j * 128 : (j + 1) * 128], ident)
                    nc.vector.tensor_copy(dstT[j][:, m * 128 : (m + 1) * 128], pt)

            # ---- support-side projections: K (LN'd, feature-major) and V ----
            kT_loc = [
                ktl.tile([128, NL], bf16, name=f"kT{j}", tag="ktl")
                for j in range(NOT)
            ]
            for m in range(NMT):
                ps_k = psA.tile([128, O], f32, name="ps_k", tag="psA")
                ps_v = psA.tile([128, O], f32, name="ps_v", tag="psA")
                for k in range(NDT):
                    lhs = sTt[k][:, m * 128 : (m + 1) * 128]
                    nc.tensor.matmul(
                        ps_k, lhs, wt[k][:, O : 2 * O],
                        start=(k == 0), stop=(k == NDT - 1),
                    )
                    nc.tensor.matmul(
                        ps_v, lhs, wt[k][:, 2 * O : 3 * O],
                        start=(k == 0), stop=(k == NDT - 1),
                    )
                ln_transpose(ps_k, bk_sb, kT_loc, m)
                v_t = vl.tile([128, O], bf16, name="v_t", tag="vl")
                nc.vector.tensor_add(v_t, ps_v, bv_sb)
                dst = cc_in[KSZ + m * 128 * O : KSZ + (m + 1) * 128 * O]
                nc.sync.dma_start(out=dst.rearrange("(p f) -> p f", p=128), in_=v_t)
            for j in range(NOT):
                dst = cc_in[j * 128 * NL : (j + 1) * 128 * NL]
                nc.sync.dma_start(
                    out=dst.rearrange("(p f) -> p f", p=128), in_=kT_loc[j]
                )

            nc.gpsimd.collective_compute(
                "AllGather",
                Alu.bypass,
                replica_groups=[list(range(NCORES))],
                ins=[cc_in.opt()],
                outs=[cc_out.opt()],
            )

            # ---- query-side projections (overlap the AllGather) ----
            qTt = []
            for k in range(NDT):
                qtk = actp.tile([128, NL], bf16, name=f"qTt{k}", tag="act")
                nc.sync.dma_start(out=qtk, in_=qT[k * 128 : (k + 1) * 128, :])
                qTt.append(qtk)
            qqT = [
                qqp.tile([128, NL], bf16, name=f"qqT{j}", tag="qq")
                for j in range(NOT)
            ]
            for m in range(NMT):
                ps_q = psA.tile([128, O], f32, name="ps_q", tag="psA")
                ps_pv = psA.tile([128, O], f32, name="ps_pv", tag="psA")
                for k in range(NDT):
                    lhs = qTt[k][:, m * 128 : (m + 1) * 128]
                    nc.tensor.matmul(
                        ps_q, lhs, wt[k][:, 0:O],
                        start=(k == 0), stop=(k == NDT - 1),
                    )
                    nc.tensor.matmul(
                        ps_pv, lhs, wt[k][:, 2 * O : 3 * O],
                        start=(k == 0), stop=(k == NDT - 1),
                    )
                ln_transpose(ps_q, bq_sb, qqT, m)
                qp_sb = lnp.tile([128, O], f32, name="qp_sb", tag="lnp")
                nc.vector.tensor_add(qp_sb, ps_pv, bv_sb)
                nc.sync.dma_start(out=out_q[m * 128 : (m + 1) * 128, :], in_=qp_sb)

            # ---- attention: two query halves of 512, streaming K/V blocks ----
            for qh in range(2):
                sums_ps = psS.tile([128, 4], f32, name="sums_ps", tag="psS")
                av_ps = [
                    psA.tile([128, O], f32, name=f"av{qi}", tag="psA")
                    for qi in range(4)
                ]
                for r in range(NCORES):
                    base = r * NL * D
                    kts = []
                    for j in range(NOT):
                        kst = ksp.tile([128, NL], bf16, name="kst", tag="ks")
                        src = cc_out[base + j * 128 * NL : base + (j + 1) * 128 * NL]
                        nc.sync.dma_start(
                            out=kst, in_=src.rearrange("(p f) -> p f", p=128)
                        )
                        kts.append(kst)
                    vts = []
                    for tl in range(NMT):
                        vst = vsp.tile([128, O], bf16, name="vst", tag="vs")
                        src = cc_out[
                            base + KSZ + tl * 128 * O : base + KSZ + (tl + 1) * 128 * O
                        ]
                        nc.sync.dma_start(
                            out=vst, in_=src.rearrange("(p f) -> p f", p=128)
                        )
                        vts.append(vst)
                    for tl in range(NMT):
                        t_abs = r * NMT + tl
                        sc = psB.tile([128, O], f32, name="sc", tag="psB")
                        for j in range(NOT):
                            nc.tensor.matmul(
                                sc,
                                kts[j][:, tl * 128 : (tl + 1) * 128],
                                qqT[j][:, qh * O : (qh + 1) * O],
                                start=(j == 0),
                                stop=(j == NOT - 1),
                            )
                        ex = exl.tile([128, O], bf16, name="ex", tag="exl")
                        nc.scalar.activation(ex, sc, Act.Exp, scale=SCALE)
                        for qi in range(4):
                            exq = ex[:, qi * 128 : (qi + 1) * 128]
                            nc.tensor.matmul(
                                av_ps[qi], exq, vts[tl],
                                start=(t_abs == 0), stop=(t_abs == NST - 1),
                            )
                            nc.tensor.matmul(
                                sums_ps[:, qi : qi + 1], exq, ones,
                                start=(t_abs == 0), stop=(t_abs == NST - 1),
                            )
                rec = stp.tile([128, 4], f32, name="rec", tag="stp")
                nc.vector.reciprocal(rec, sums_ps)
                for qi in range(4):
                    oc = ocp.tile([128, O], f32, name="oc", tag="ocp")
                    nc.vector.tensor_scalar_mul(oc, av_ps[qi], rec[:, qi : qi + 1])
                    row = (qh * 4 + qi) * 128
                    nc.sync.dma_start(out=out_c[row : row + 128, :], in_=oc)

    nc.compile()
    return nc


def _prep_inputs(support_set, queries, Wq, bq, Wk, bk, Wv, bv, ln_gamma, ln_beta):
    sT_full = np.ascontiguousarray(np.asarray(support_set, np.float32).T).astype(BF16)
    qT_full = np.ascontiguousarray(np.asarray(queries, np.float32).T).astype(BF16)
    w_cat = np.ascontiguousarray(
        np.concatenate(
            [np.asarray(Wq).T, np.asarray(Wk).T, np.asarray(Wv).T], axis=1
        ).astype(np.float32)
    ).astype(BF16)

    def bc(v):
        return np.ascontiguousarray(
            np.broadcast_to(np.asarray(v, np.float32)[None, :], (128, O))
        )

    shared = {
        "w": w_cat,
        "bq_b": bc(bq),
        "bk_b": bc(bk),
        "bv_b": bc(bv),
        "g_b": bc(ln_gamma),
        "be_b": bc(ln_beta),
    }
    in_maps = []
    for i in range(NCORES):
        m = dict(shared)
        m["sT"] = np.ascontiguousarray(sT_full[:, i * NL : (i + 1) * NL])
        m["qT"] = np.ascontiguousarray(qT_full[:, i * NL : (i + 1) * NL])
        in_maps.append(m)
    return in_maps


def kernel(support_set, queries, Wq, bq, Wk, bk, Wv, bv, ln_gamma, ln_beta):
    global LAST_RESULTS
    from concourse.bass_utils import run_bass_kernel_spmd

    if "nc" not in _CACHE:
        _CACHE["nc"] = _build_graph()
    nc = _CACHE["nc"]

    in_maps = _prep_inputs(
        support_set, queries, Wq, bq, Wk, bk, Wv, bv, ln_gamma, ln_beta
    )
    _CACHE["in_maps"] = in_maps
    res = run_bass_kernel_spmd(
        nc, in_maps, core_ids=list(range(NCORES)), trace=False
    )
    LAST_RESULTS = res
    q_proto = np.concatenate([res.results[i]["out_q"] for i in range(NCORES)], axis=0)
    c_proto = np.concatenate([res.results[i]["out_c"] for i in range(NCORES)], axis=0)
    return (
        np.asarray(q_proto, np.float32),
        np.asarray(c_proto, np.float32),
    )


def _bench_callable(n_iters):
    """A jitted callable that executes the NEFF n_iters times back-to-back
    across all 8 cores.  Timing (t(N) - t(1)) / (N - 1) isolates per-execution
    device time from fixed dispatch/transfer overhead."""
    import jax
    from jax.experimental.shard_map import shard_map
    from jax.sharding import Mesh, PartitionSpec

    from concourse import bass2jax, mybir

    nc = _CACHE["nc"]
    in_maps = _CACHE["in_maps"]

    partition_name = (
        nc.partition_id_tensor.name if nc.partition_id_tensor else None
    )
    in_names: list[str] = []
    out_names: list[str] = []
    out_avals = []
    zero_outs = []
    for alloc in nc.m.functions[0].allocations:
        if not isinstance(alloc, mybir.MemoryLocationSet):
            continue
        name = alloc.memorylocations[0].name
        if alloc.kind == "ExternalInput":
            if name != partition_name:
                in_names.append(name)
        elif alloc.kind == "ExternalOutput":
            shape = tuple(alloc.tensor_shape)
            dtype = mybir.dt.np(alloc.dtype)
            out_names.append(name)
            out_avals.append(jax.core.ShapedArray(shape, dtype))
            zero_outs.append(np.zeros(shape, dtype))
    n_params = len(in_names)
    in_names_full = list(in_names) + out_names
    if partition_name is not None:
        in_names_full.append(partition_name)

    def _body(*args):
        operands = list(args)
        if partition_name is not None:
            operands.append(bass2jax.partition_id_tensor())
        outs = None
        for _ in range(n_iters):
            outs = bass2jax._bass_exec_p.bind(
                *operands,
                out_avals=tuple(out_avals),
                in_names=tuple(in_names_full),
                out_names=tuple(out_names),
                lowering_input_output_aliases=(),
                sim_require_finite=True,
                sim_require_nnan=True,
                nc=nc,
            )
        return tuple(outs)

    devices = jax.devices()[:NCORES]
    mesh = Mesh(np.asarray(devices), ("core",))
    n_outs = len(out_avals)
    in_specs = (PartitionSpec("core"),) * (n_params + n_outs)
    out_specs = (PartitionSpec("core"),) * n_outs
    sharded = jax.jit(
        shard_map(
            _body, mesh=mesh, in_specs=in_specs, out_specs=out_specs,
            check_rep=False,
        )
    )
    per_core = [
        [np.asarray(in_maps[c][name]) for name in in_names] for c in range(NCORES)
    ]
    concat_in = [
        np.concatenate([per_core[c][i] for c in range(NCORES)], axis=0)
        for i in range(n_params)
    ]
    concat_zeros = [
        np.zeros((NCORES * z.shape[0], *z.shape[1:]), z.dtype) for z in zero_outs
    ]

    def run():
        out = sharded(*concat_in, *concat_zeros)
        jax.block_until_ready(out)
        return out

    return run


def benchmark(n_iters=17, reps=4):
    """Returns estimated per-execution device time in ns."""
    import time

    assert "nc" in _CACHE and "in_maps" in _CACHE, "call kernel() first"
    run1 = _bench_callable(1)
    runN = _bench_callable(n_iters)
    run1()
    runN()  # warm both compiles

    def med(fn, k):
        ts = []
        for _ in range(k):
            t0 = time.perf_counter()
            fn()
            ts.append(time.perf_counter() - t0)
        return float(np.median(ts))

    t1 = med(run1, reps)
    tN = med(runN, reps)
    per_exec_s = (tN - t1) / (n_iters - 1)
    return per_exec_s * 1e9, t1, tN
